# revision 32
# baseline (speedup 1.0000x reference)
"""Trainium2 Bass kernel: causal self-attention with RoPE (16 heads, B=2, S=2048, D=2048).

Sharding: 8 cores = 2 (batch, data-parallel) x 4 (head-groups of 4 heads, tensor
parallel).  Each core computes q/k/v projections for its 4 heads, RoPE, causal
attention, and a partial o_proj over its 512 rows of Wo's contraction dim.
The 4 partial [S, D] outputs per batch are summed on the host (the "all-reduce"
of the o_proj), which is part of the unshard/gather step.

v2 design (all-bf16 data path):
  - inputs pre-cast to bf16 on host; all SBUF tensors bf16, PSUM f32.
  - weights fully resident in SBUF (loaded once, no per-window re-DMA).
  - v written straight from PSUM into per-head SBUF column tiles (no DRAM
    spill round-trip; the old gather never crossed partitions).
  - causal mask applied as a post-exp 0/1 multiply on the vector engine
    (removes 64 mask matmuls from the tensor stream).
  - attention emission software-pipelined: scores run 2 j-blocks ahead of
    the pv/cs matmuls so the tensor engine never waits on the scalar exp.
  - o_proj interleaved per query-group (no separate phase tail).
  - softmax denominators via reciprocal_approx_fast (f32, ~18 bits).
"""

import math

import numpy as np

# ---- problem constants ----
B, S, D = 2, 2048, 2048
NUM_HEADS, HD = 16, 128
N_CORES = 8
GROUPS = 4                  # head-groups (tensor-parallel)
H_PER_CORE = NUM_HEADS // GROUPS   # 4
E_PER_CORE = H_PER_CORE * HD       # 512

NEG_BIG = -1.0e30

_CACHE = {}


# --------------------------------------------------------------------------
# host-side helpers
# --------------------------------------------------------------------------

def _rope_sin_cos(seq_len, head_dim):
    """float32, matches reference._rope_sin_cos."""
    pos = np.arange(seq_len, dtype=np.float32)
    freq_seq = np.arange(0, head_dim, 2, dtype=np.float32)
    inv_freq = (np.float32(1.0) / (np.float32(10000.0) ** (freq_seq / np.float32(head_dim)))).astype(np.float32)
    sinusoid = pos[:, None] * inv_freq[None, :]          # [S, hd/2]
    return np.sin(sinusoid).astype(np.float32), np.cos(sinusoid).astype(np.float32)


def _rope_tables(seq_len):
    """CC / SS' [128, seq_len] f32 in the quadrant-paired layout.
    CC row = cos(pair angle) at both x1 and x2 rows.
    SS' = +sin at x1 rows, -sin at x2 rows, so that
    shuffle16(ps*SS') = [-x2*sin at x1 rows ; x1*sin at x2 rows]."""
    sin, cos = _rope_sin_cos(seq_len, HD)       # [S, 64]
    cosT = cos.T                                # [64, S] pair-index order
    sinT = sin.T
    x1, x2 = _pair_pos()
    CC = np.empty((HD, seq_len), dtype=np.float32)
    SS = np.empty((HD, seq_len), dtype=np.float32)
    CC[x1] = cosT
    CC[x2] = cosT
    SS[x1] = sinT
    SS[x2] = -sinT
    return CC, SS


def _deinterleave_idx():
    """Row permutation within one head: quadrant-paired so the RoPE partner
    swap is a within-quadrant rotation by 16 (DVE stream_shuffle-able)."""
    idx = np.empty(HD, dtype=np.int64)
    for q in range(4):
        t = 16 * q + np.arange(16)
        idx[q * 32:q * 32 + 16] = 2 * t
        idx[q * 32 + 16:q * 32 + 32] = 2 * t + 1
    return idx


def _pair_pos():
    """(x1_rows, x2_rows) in the deinterleaved layout, pair-index order."""
    x1 = np.concatenate([q * 32 + np.arange(16) for q in range(4)])
    x2 = x1 + 16
    return x1, x2


def _mask_strip():
    """[128, 128] additive causal mask for the diagonal boundary strip:
    m[jr, c] = 0 if jr <= c else NEG_BIG."""
    jr = np.arange(128)[:, None]
    c = np.arange(128)[None, :]
    return np.where(jr <= c, 0.0, NEG_BIG).astype(np.float32)


def _np_rope_apply(q, sin, cos):
    """q: [S, 128] in the quadrant-paired deinterleaved layout."""
    p1, p2 = _pair_pos()
    x1, x2 = q[:, p1], q[:, p2]
    r = np.empty_like(q)
    r[:, p1] = x1 * cos - x2 * sin
    r[:, p2] = x1 * sin + x2 * cos
    return r


def _np_core_model(xT, wq, wk, wv, wo):
    """Numpy model of what ONE core's device program computes (f32)."""
    Dm, S_ = xT.shape
    E_ = wq.shape[1]
    H_ = E_ // HD
    x = xT.T.astype(np.float32)
    sin, cos = _rope_sin_cos(S_, HD)
    out = np.zeros((S_, Dm), dtype=np.float32)
    causal = np.tril(np.ones((S_, S_), dtype=bool))
    for h in range(H_):
        q = x @ wq[:, h * HD:(h + 1) * HD].astype(np.float32)
        k = x @ wk[:, h * HD:(h + 1) * HD].astype(np.float32)
        v = x @ wv[:, h * HD:(h + 1) * HD].astype(np.float32)
        q = _np_rope_apply(q, sin, cos)
        k = _np_rope_apply(k, sin, cos)
        s = (q @ k.T) / math.sqrt(HD)
        s = np.where(causal, s, -np.inf)
        p = np.exp(s - s.max(axis=-1, keepdims=True))
        p = p / p.sum(axis=-1, keepdims=True)
        out += (p @ v) @ wo[h * HD:(h + 1) * HD, :].astype(np.float32)
    return out


def _np_reference(x, Wq, Wk, Wv, Wo, attn_mask):
    """Full-problem numpy fallback replicating reference.py (generic mask)."""
    B_, S_, D_ = x.shape
    H = NUM_HEADS
    hd = D_ // H
    sin, cos = _rope_sin_cos(S_, hd)

    def proj(W):
        y = np.einsum('bsd,ed->bse', x, W)
        return y.reshape(B_, S_, H, hd).transpose(0, 2, 1, 3)

    q, k, v = proj(Wq), proj(Wk), proj(Wv)

    def rope(t):
        tr = t.reshape(B_, H, S_, hd // 2, 2)
        x1, x2 = tr[..., 0], tr[..., 1]
        r1 = x1 * cos[None, None] - x2 * sin[None, None]
        r2 = x1 * sin[None, None] + x2 * cos[None, None]
        return np.stack((r1, r2), axis=-1).reshape(B_, H, S_, hd)

    q, k = rope(q), rope(k)
    scores = np.einsum('bhqd,bhkd->bhqk', q, k) / math.sqrt(hd) + attn_mask
    scores = scores - scores.max(axis=-1, keepdims=True)
    p = np.exp(scores)
    p = p / p.sum(axis=-1, keepdims=True)
    attn = np.einsum('bhqk,bhkd->bhqd', p, v)
    attn = attn.transpose(0, 2, 1, 3).reshape(B_, S_, D_)
    return np.einsum('bsd,ed->bse', attn, Wo)


# --------------------------------------------------------------------------
# device program builder (v2, all-bf16)
# --------------------------------------------------------------------------

def build(S_=S, Dm_=D, H_=H_PER_CORE):
    """Build the per-core Bass program (SPMD: same program, 8 data shards).

    Inputs (DRAM, bf16):  xT [Dm_, S_], wqk [Dm_, 2*E_], wv [Dm_, E_], wo [E_, Dm_]
    Output (DRAM, bf16):  out [S_, Dm_]   (partial o_proj; host sums groups)
    """
    import concourse.bass as bass
    import concourse.tile as tile
    from concourse import bacc, mybir
    import ml_dtypes

    f32 = mybir.dt.float32
    bf16 = mybir.dt.bfloat16
    AF = mybir.ActivationFunctionType

    E_ = H_ * HD
    DT = Dm_ // 128            # contraction tiles (16)
    W = min(1024, S_)          # qkv s-window
    NW = S_ // W               # 2
    NSW = W // 512             # 512-swaths per window (2)
    NG = S_ // 512             # attention query groups (4)
    VB = W // 128              # v s-blocks per window (8)
    SBK = S_ // 128            # total s-blocks (16)
    NDB = Dm_ // 512           # o_proj D chunks (4)
    SCALE = 1.0 / math.sqrt(HD)
    SWAP16 = [(i + 16) % 32 for i in range(32)]
    NHP = max(1, H_ // 2)      # head pairs

    nc = bacc.Bacc("TRN2", target_bir_lowering=False, debug=False)

    xT_d = nc.dram_tensor("xT", [Dm_, S_], bf16, kind="ExternalInput")
    wqk_d = nc.dram_tensor("wqk", [Dm_, 2 * E_], bf16, kind="ExternalInput")
    wv_d = nc.dram_tensor("wv", [Dm_, E_], bf16, kind="ExternalInput")
    wo_d = nc.dram_tensor("wo", [E_, Dm_], bf16, kind="ExternalInput")
    out_d = nc.dram_tensor("out", [S_, Dm_], bf16, kind="ExternalOutput")

    CC_np, SS_np = _rope_tables(S_)
    cc_dram = nc.inline_tensor(CC_np.astype(ml_dtypes.bfloat16), "cc_const")
    ss_dram = nc.inline_tensor(SS_np.astype(ml_dtypes.bfloat16), "ss_const")
    mask_dram = nc.inline_tensor(_mask_strip().astype(ml_dtypes.bfloat16), "mask_const")
    ones_dram = nc.inline_tensor(np.ones((128, 128), dtype=ml_dtypes.bfloat16), "ones_const")
    ident_dram = nc.inline_tensor(np.eye(128, dtype=ml_dtypes.bfloat16), "ident_const")

    from contextlib import ExitStack

    with tile.TileContext(nc) as tc, ExitStack() as ctx:
        # ---- persistent pools (stack allocator: order matters) ----
        cpool = ctx.enter_context(tc.tile_pool(name="consts", bufs=1))
        qkpool = ctx.enter_context(tc.tile_pool(name="qkT", bufs=1))

        # mask/ones/ident allocated here but DMA'd at phase-2 open (sync queue
        # is needed for x tiles first; these aren't read until ~190us in)
        mask = cpool.tile([128, 128], bf16, tag="mask", name="mask")
        ones = cpool.tile([128, 128], bf16, tag="ones", name="ones")
        ident = cpool.tile([128, 128], bf16, tag="ident", name="ident")

        # persistent bf16 tensors: qkT[h][p] [128, S], v columns per head [128, S]
        qkT = [[qkpool.tile([128, S_], bf16, tag=f"qk{h}_{p}", name=f"qk{h}_{p}")
                for p in range(2)] for h in range(H_)]
        vch = [qkpool.tile([128, S_], bf16, tag=f"vch{h}", name=f"vch{h}")
               for h in range(H_)]

        # ---------------- phase QKV (one pass over xT windows) ----------------
        with tc.tile_pool(name="wres", bufs=1) as wrpool, \
             tc.tile_pool(name="xt", bufs=2 * DT) as xtpool, \
             tc.tile_pool(name="rsw", bufs=5) as rswpool, \
             tc.tile_pool(name="ropew", bufs=1) as rpool, \
             tc.tile_pool(name="qkps", bufs=8, space="PSUM") as qkvps:

            # resident weights + rope tables (single load); wv on the scalar
            # queue and wqk on the gpsimd queue so both stream concurrently
            # with the x tiles on the sync queue.
            # DMA queues: x tiles split sync(even d)/scalar(odd d); weights on
            # gpsimd (wv first -- needed from ~8us); rope tables on scalar
            # after window-0 x. Per-queue transfers serialize, so order ==
            # need-by time.
            wvt = []
            for d in range(DT):
                t = wrpool.tile([128, E_], bf16, tag=f"wv{d}", name=f"wv{d}")
                nc.scalar.dma_start(t[:], wv_d[d * 128:(d + 1) * 128, :])
                wvt.append(t)
            xts_w = [[xtpool.tile([128, W], bf16, tag="xt", name="xt")
                      for _ in range(DT)] for _ in range(NW)]
            for d in range(DT):
                eng = nc.sync if d % 2 == 0 else nc.gpsimd
                eng.dma_start(xts_w[0][d][:], xT_d[d * 128:(d + 1) * 128, 0:W])
            wqkt = []
            for d in range(DT):
                t = wrpool.tile([128, 2 * E_], bf16, tag=f"wqk{d}", name=f"wqk{d}")
                nc.gpsimd.dma_start(t[:], wqk_d[d * 128:(d + 1) * 128, :])
                wqkt.append(t)
            cc = rpool.tile([128, S_], bf16, tag="cc", name="cc")
            nc.scalar.dma_start(cc[:], cc_dram[:])
            ss = rpool.tile([128, S_], bf16, tag="ss", name="ss")
            nc.scalar.dma_start(ss[:], ss_dram[:])
            for win in range(1, NW):
                for d in range(DT):
                    eng = nc.sync if d % 2 == 0 else nc.gpsimd
                    eng.dma_start(xts_w[win][d][:],
                                  xT_d[d * 128:(d + 1) * 128, win * W:(win + 1) * W])

            # warm-up matmuls: the tensor engine would idle ~8us waiting for
            # the first x/wv tiles; dependency-free dummy matmuls (scratch
            # SBUF, result never read) ramp the PE p-state and HAM credit so
            # real matmuls start at full clock.
            wu_src = rswpool.tile([128, 512], bf16, tag="wu", name="wu")
            nc.gpsimd.memset(wu_src[:], 0)
            wu_ps = qkvps.tile([128, E_], f32, tag="psqk", name="wups")
            for _ in range(36):
                nc.tensor.matmul(wu_ps[:], wu_src[:, 0:128], wu_src[:],
                                 start=True, stop=True, skip_group_check=True)

            for win in range(NW):
                xts = xts_w[win]

                # ---- v for this window: d-outer (rides DMA arrival), VB banks ----
                vps_t = [qkvps.tile([128, E_], f32, tag="psqk", name="psv") for _ in range(VB)]
                for d in range(DT):
                    for vb in range(VB):
                        nc.tensor.matmul(vps_t[vb][:], xts[d][:, vb * 128:(vb + 1) * 128],
                                         wvt[d][:],
                                         start=(d == 0), stop=(d == DT - 1))
                for vb in range(VB):
                    sb = win * VB + vb
                    for h in range(H_):
                        nc.scalar.copy(vch[h][:, sb * 128:(sb + 1) * 128],
                                       vps_t[vb][:, h * 128:(h + 1) * 128])

                # ---- q/k for this window: 4 psum tiles per (head-pair, q|k) ----
                for hp in range(NHP):
                    nh = min(2, H_ - 2 * hp)
                    for p in range(2):
                        pss = [[qkvps.tile([128, 512], f32, tag="psqk", name="psqk")
                                for _ in range(NSW)] for _ in range(nh)]
                        c0w = hp * 512 + p * 128 * nh
                        for d in range(DT):
                            for h2 in range(nh):
                                for sw in range(NSW):
                                    nc.tensor.matmul(
                                        pss[h2][sw][:],
                                        wqkt[d][:, c0w + h2 * 128:c0w + (h2 + 1) * 128],
                                        xts[d][:, sw * 512:(sw + 1) * 512],
                                        start=(d == 0), stop=(d == DT - 1))
                        raws = []
                        for h2 in range(nh):
                            for sw in range(NSW):
                                raw = rswpool.tile([128, 512], bf16, tag="raw", name="raw")
                                nc.scalar.copy(raw[:], pss[h2][sw][:])
                                raws.append((h2, sw, raw))
                        for h2, sw, raw in raws:
                            h = 2 * hp + h2
                            cw = win * W + sw * 512
                            dst = qkT[h][p]
                            m1 = rswpool.tile([128, 512], bf16, tag="m1", name="m1", bufs=2)
                            m2 = rswpool.tile([128, 512], bf16, tag="m2", name="m2", bufs=2)
                            nc.vector.tensor_mul(m1[:], raw[:], cc[:, cw:cw + 512])
                            nc.vector.tensor_mul(m2[:], raw[:], ss[:, cw:cw + 512])
                            m2s = rswpool.tile([128, 512], bf16, tag="m2s", name="m2s", bufs=2)
                            nc.vector.stream_shuffle(m2s[:], m2[:], mask=SWAP16)
                            nc.vector.tensor_add(dst[:, cw:cw + 512], m1[:], m2s[:])

        # ---------------- phase ATTENTION + O_PROJ (interleaved, g-outer) ----------------
        with tc.tile_pool(name="wo", bufs=1) as wopool, \
             tc.tile_pool(name="att", bufs=1) as apool, \
             tc.tile_pool(name="pt", bufs=6) as ptpool, \
             tc.tile_pool(name="stg", bufs=4) as stgpool, \
             tc.tile_pool(name="ost", bufs=6) as ostpool, \
             tc.tile_pool(name="stps", bufs=3, space="PSUM") as stps, \
             tc.tile_pool(name="pvps", bufs=2, space="PSUM") as pvps, \
             tc.tile_pool(name="csps", bufs=1, space="PSUM") as csps, \
             tc.tile_pool(name="ops", bufs=2, space="PSUM") as opsp:

            nc.sync.dma_start(mask[:], mask_dram[:])
            nc.sync.dma_start(ones[:], ones_dram[:])
            nc.sync.dma_start(ident[:], ident_dram[:])
            wot = [wopool.tile([128, Dm_], bf16, tag=f"wo{h}", name=f"wo{h}") for h in range(H_)]
            for h in range(H_):
                nc.sync.dma_start(wot[h][:], wo_d[h * 128:(h + 1) * 128, :])



            for g in range(NG):
                njb = 4 * g + 4
                # attnT[h] transient per group: [128 hd, 512 q] bf16 (2 bufs
                # so group g+1 writes don't serialize on group g o_proj reads)
                attnT = [apool.tile([128, 512], bf16, tag=f"attnT{h}", name=f"attnT{h}", bufs=2)
                         for h in range(H_)]
                for h in range(H_):
                    qslice = qkT[h][0][:, g * 512:(g + 1) * 512]
                    pv = pvps.tile([128, 512], f32, tag="pv", name="pv")
                    cs = csps.tile([128, 512], f32, tag="cs", name="cs")
                    sts = [None] * njb
                    pts = [None] * njb

                    def emit_scores(jb):
                        st = stps.tile([128, 512], f32, tag="st", name="st")
                        dgi = jb - 4 * g
                        c0 = max(0, dgi) * 128  # masked cols skipped for diag blocks
                        if dgi >= 0:
                            # scores on the live columns, then the additive
                            # -1e30 causal strip on the 128-col boundary
                            nc.tensor.matmul(st[:, c0:512],
                                             qkT[h][1][:, jb * 128:(jb + 1) * 128],
                                             qslice[:, c0:512], start=True, stop=False)
                            nc.tensor.matmul(st[:, c0:c0 + 128], ident[:], mask[:],
                                             start=False, stop=True, skip_group_check=True)
                        else:
                            nc.tensor.matmul(st[:, c0:512],
                                             qkT[h][1][:, jb * 128:(jb + 1) * 128],
                                             qslice[:, c0:512], start=True, stop=True)
                        pt = ptpool.tile([128, 512], bf16, tag="pt", name="pt")
                        if c0 > 0:
                            nc.gpsimd.memset(pt[:, 0:c0], 0)
                        nc.scalar.activation(pt[:, c0:512], st[:, c0:512], AF.Exp, scale=SCALE)
                        sts[jb] = st
                        pts[jb] = pt

                    def emit_pvcs(jb):
                        nc.tensor.matmul(pv[:], vch[h][:, jb * 128:(jb + 1) * 128],
                                         pts[jb][:], start=(jb == 0), stop=(jb == njb - 1))
                        nc.tensor.matmul(cs[:], ones[:], pts[jb][:],
                                         start=(jb == 0), stop=(jb == njb - 1))
                        pts[jb] = None

                    for jb in range(njb):
                        emit_scores(jb)
                        if jb >= 2:
                            emit_pvcs(jb - 2)
                    emit_pvcs(njb - 2)
                    emit_pvcs(njb - 1)

                    rc = stgpool.tile([128, 512], f32, tag="rc", name="rc")
                    nc.vector.reciprocal_approx_fast(rc[:], cs[:])
                    nc.vector.tensor_mul(attnT[h][:], pv[:], rc[:])

                # o_proj for this group's 4 s-blocks
                for sbr in range(4):
                    sb = 4 * g + sbr
                    for db in range(NDB):
                        op = opsp.tile([128, 512], f32, tag="ops", name="ops")
                        for h in range(H_):
                            nc.tensor.matmul(op[:],
                                             attnT[h][:, sbr * 128:(sbr + 1) * 128],
                                             wot[h][:, db * 512:(db + 1) * 512],
                                             start=(h == 0), stop=(h == H_ - 1))
                        o = ostpool.tile([128, 512], bf16, tag="ost", name="ost")
                        nc.vector.tensor_copy(o[:], op[:])
                        nc.sync.dma_start(
                            out_d[sb * 128:(sb + 1) * 128, db * 512:(db + 1) * 512], o[:])

    nc.compile()
    return nc


# --------------------------------------------------------------------------
# host sharding + entry point
# --------------------------------------------------------------------------

def _prep_core_inputs(x, Wq, Wk, Wv, Wo, fp32r=False):
    """Return list of 8 per-core input dicts (bf16)."""
    import ml_dtypes
    perm = _deinterleave_idx()
    in_maps = []
    for c in range(N_CORES):
        b, g = c // GROUPS, c % GROUPS
        heads = range(g * H_PER_CORE, (g + 1) * H_PER_CORE)
        qk_rows = np.concatenate([h * HD + perm for h in heads])
        v_rows = np.concatenate([np.arange(h * HD, (h + 1) * HD) for h in heads])
        wq_t = Wq[qk_rows, :].T
        wk_t = Wk[qk_rows, :].T
        E_ = len(qk_rows)
        wqk = np.empty((Wq.shape[1], 2 * E_), dtype=np.float32)
        for hp in range((E_ // HD + 1) // 2):
            nh = min(2, E_ // HD - 2 * hp)
            cq = 256 * hp
            wqk[:, 2 * cq:2 * cq + nh * 128] = wq_t[:, cq:cq + nh * 128]
            wqk[:, 2 * cq + nh * 128:2 * cq + 2 * nh * 128] = wk_t[:, cq:cq + nh * 128]
        bf = ml_dtypes.bfloat16
        in_maps.append({
            "xT": np.ascontiguousarray(x[b].T).astype(bf),
            "wqk": np.ascontiguousarray(wqk).astype(bf),
            "wv": np.ascontiguousarray(Wv[v_rows, :].T).astype(bf),
            "wo": np.ascontiguousarray(Wo[:, v_rows].T).astype(bf),
        })
    return in_maps


def _is_causal_mask(attn_mask):
    if attn_mask is None:
        return True
    m = np.asarray(attn_mask)
    if m.shape != (1, 1, S, S):
        return False
    m2 = m[0, 0]
    tril = np.tril(np.ones((S, S), dtype=bool))
    return bool(np.all(m2[tril] == 0.0) and np.all(m2[~tril] <= -1.0e30))


def _get_program(mmdt="bf16"):
    key = ("v2", "bf16")
    if key not in _CACHE:
        _CACHE[key] = build(S, D, H_PER_CORE)
    return _CACHE[key]


def run_on_hw(in_maps, mmdt="bf16", trace=False, **kwargs):
    """Run the SPMD program on the 8 NeuronCores; returns BassKernelResults."""
    from concourse.bass_utils import run_bass_kernel_spmd
    nc = _get_program(mmdt)
    return run_bass_kernel_spmd(nc, in_maps, core_ids=list(range(N_CORES)),
                                trace=trace, **kwargs)


def kernel(x, Wq, Wk, Wv, Wo, attn_mask=None, **_ignored):
    x = np.asarray(x, dtype=np.float32)
    Wq = np.asarray(Wq, dtype=np.float32)
    Wk = np.asarray(Wk, dtype=np.float32)
    Wv = np.asarray(Wv, dtype=np.float32)
    Wo = np.asarray(Wo, dtype=np.float32)

    if not _is_causal_mask(attn_mask):
        return _np_reference(x, Wq, Wk, Wv, Wo, np.asarray(attn_mask, dtype=np.float32)).astype(np.float32)

    in_maps = _prep_core_inputs(x, Wq, Wk, Wv, Wo)
    res = run_on_hw(in_maps, trace=False)

    out = np.zeros((B, S, D), dtype=np.float32)
    for c in range(N_CORES):
        out[c // GROUPS] += res.results[c]["out"].astype(np.float32)
    return out


# revision 34
# speedup vs baseline: 1.0951x; 1.0951x over previous
"""Trainium2 Bass kernel: causal self-attention with RoPE (16 heads, B=2, S=2048, D=2048).

Sharding: 8 cores = 2 (batch, data-parallel) x 4 (head-groups of 4 heads, tensor
parallel).  Each core computes q/k/v projections for its 4 heads, RoPE, causal
attention, and a partial o_proj over its 512 rows of Wo's contraction dim.
The 4 partial [S, D] outputs per batch are summed on the host (the "all-reduce"
of the o_proj), which is part of the unshard/gather step.

v2 design (all-bf16 data path):
  - inputs pre-cast to bf16 on host; all SBUF tensors bf16, PSUM f32.
  - weights fully resident in SBUF (loaded once, no per-window re-DMA).
  - v written straight from PSUM into per-head SBUF column tiles (no DRAM
    spill round-trip; the old gather never crossed partitions).
  - causal mask applied as a post-exp 0/1 multiply on the vector engine
    (removes 64 mask matmuls from the tensor stream).
  - attention emission software-pipelined: scores run 2 j-blocks ahead of
    the pv/cs matmuls so the tensor engine never waits on the scalar exp.
  - o_proj interleaved per query-group (no separate phase tail).
  - softmax denominators via reciprocal_approx_fast (f32, ~18 bits).
"""

import math

import numpy as np

# ---- problem constants ----
B, S, D = 2, 2048, 2048
NUM_HEADS, HD = 16, 128
N_CORES = 8
GROUPS = 4                  # head-groups (tensor-parallel)
H_PER_CORE = NUM_HEADS // GROUPS   # 4
E_PER_CORE = H_PER_CORE * HD       # 512

NEG_BIG = -1.0e30

_CACHE = {}


# --------------------------------------------------------------------------
# host-side helpers
# --------------------------------------------------------------------------

def _rope_sin_cos(seq_len, head_dim):
    """float32, matches reference._rope_sin_cos."""
    pos = np.arange(seq_len, dtype=np.float32)
    freq_seq = np.arange(0, head_dim, 2, dtype=np.float32)
    inv_freq = (np.float32(1.0) / (np.float32(10000.0) ** (freq_seq / np.float32(head_dim)))).astype(np.float32)
    sinusoid = pos[:, None] * inv_freq[None, :]          # [S, hd/2]
    return np.sin(sinusoid).astype(np.float32), np.cos(sinusoid).astype(np.float32)


def _rope_tables(seq_len):
    """CC / SS' [128, seq_len] f32 in the quadrant-paired layout.
    CC row = cos(pair angle) at both x1 and x2 rows.
    SS' = +sin at x1 rows, -sin at x2 rows, so that
    shuffle16(ps*SS') = [-x2*sin at x1 rows ; x1*sin at x2 rows]."""
    sin, cos = _rope_sin_cos(seq_len, HD)       # [S, 64]
    cosT = cos.T                                # [64, S] pair-index order
    sinT = sin.T
    x1, x2 = _pair_pos()
    CC = np.empty((HD, seq_len), dtype=np.float32)
    SS = np.empty((HD, seq_len), dtype=np.float32)
    CC[x1] = cosT
    CC[x2] = cosT
    SS[x1] = sinT
    SS[x2] = -sinT
    return CC, SS


def _deinterleave_idx():
    """Row permutation within one head: quadrant-paired so the RoPE partner
    swap is a within-quadrant rotation by 16 (DVE stream_shuffle-able)."""
    idx = np.empty(HD, dtype=np.int64)
    for q in range(4):
        t = 16 * q + np.arange(16)
        idx[q * 32:q * 32 + 16] = 2 * t
        idx[q * 32 + 16:q * 32 + 32] = 2 * t + 1
    return idx


def _pair_pos():
    """(x1_rows, x2_rows) in the deinterleaved layout, pair-index order."""
    x1 = np.concatenate([q * 32 + np.arange(16) for q in range(4)])
    x2 = x1 + 16
    return x1, x2


def _mask_strip():
    """[128, 128] additive causal mask for the diagonal boundary strip:
    m[jr, c] = 0 if jr <= c else NEG_BIG."""
    jr = np.arange(128)[:, None]
    c = np.arange(128)[None, :]
    return np.where(jr <= c, 0.0, NEG_BIG).astype(np.float32)


def _np_rope_apply(q, sin, cos):
    """q: [S, 128] in the quadrant-paired deinterleaved layout."""
    p1, p2 = _pair_pos()
    x1, x2 = q[:, p1], q[:, p2]
    r = np.empty_like(q)
    r[:, p1] = x1 * cos - x2 * sin
    r[:, p2] = x1 * sin + x2 * cos
    return r


def _np_core_model(xT, wq, wk, wv, wo):
    """Numpy model of what ONE core's device program computes (f32)."""
    Dm, S_ = xT.shape
    E_ = wq.shape[1]
    H_ = E_ // HD
    x = xT.T.astype(np.float32)
    sin, cos = _rope_sin_cos(S_, HD)
    out = np.zeros((S_, Dm), dtype=np.float32)
    causal = np.tril(np.ones((S_, S_), dtype=bool))
    for h in range(H_):
        q = x @ wq[:, h * HD:(h + 1) * HD].astype(np.float32)
        k = x @ wk[:, h * HD:(h + 1) * HD].astype(np.float32)
        v = x @ wv[:, h * HD:(h + 1) * HD].astype(np.float32)
        q = _np_rope_apply(q, sin, cos)
        k = _np_rope_apply(k, sin, cos)
        s = (q @ k.T) / math.sqrt(HD)
        s = np.where(causal, s, -np.inf)
        p = np.exp(s - s.max(axis=-1, keepdims=True))
        p = p / p.sum(axis=-1, keepdims=True)
        out += (p @ v) @ wo[h * HD:(h + 1) * HD, :].astype(np.float32)
    return out


def _np_reference(x, Wq, Wk, Wv, Wo, attn_mask):
    """Full-problem numpy fallback replicating reference.py (generic mask)."""
    B_, S_, D_ = x.shape
    H = NUM_HEADS
    hd = D_ // H
    sin, cos = _rope_sin_cos(S_, hd)

    def proj(W):
        y = np.einsum('bsd,ed->bse', x, W)
        return y.reshape(B_, S_, H, hd).transpose(0, 2, 1, 3)

    q, k, v = proj(Wq), proj(Wk), proj(Wv)

    def rope(t):
        tr = t.reshape(B_, H, S_, hd // 2, 2)
        x1, x2 = tr[..., 0], tr[..., 1]
        r1 = x1 * cos[None, None] - x2 * sin[None, None]
        r2 = x1 * sin[None, None] + x2 * cos[None, None]
        return np.stack((r1, r2), axis=-1).reshape(B_, H, S_, hd)

    q, k = rope(q), rope(k)
    scores = np.einsum('bhqd,bhkd->bhqk', q, k) / math.sqrt(hd) + attn_mask
    scores = scores - scores.max(axis=-1, keepdims=True)
    p = np.exp(scores)
    p = p / p.sum(axis=-1, keepdims=True)
    attn = np.einsum('bhqk,bhkd->bhqd', p, v)
    attn = attn.transpose(0, 2, 1, 3).reshape(B_, S_, D_)
    return np.einsum('bsd,ed->bse', attn, Wo)


# --------------------------------------------------------------------------
# device program builder (v2, all-bf16)
# --------------------------------------------------------------------------

def build(S_=S, Dm_=D, H_=H_PER_CORE):
    """Build the per-core Bass program (SPMD: same program, 8 data shards).

    Inputs (DRAM, bf16):  xT [Dm_, S_], wqk [Dm_, 2*E_], wv [Dm_, E_], wo [E_, Dm_]
    Output (DRAM, bf16):  out [S_, Dm_]   (partial o_proj; host sums groups)
    """
    import concourse.bass as bass
    import concourse.tile as tile
    from concourse import bacc, mybir
    import ml_dtypes

    f32 = mybir.dt.float32
    bf16 = mybir.dt.bfloat16
    AF = mybir.ActivationFunctionType

    E_ = H_ * HD
    DT = Dm_ // 128            # contraction tiles (16)
    W = min(1024, S_)          # qkv s-window
    NW = S_ // W               # 2
    NSW = W // 512             # 512-swaths per window (2)
    NG = S_ // 512             # attention query groups (4)
    VB = W // 128              # v s-blocks per window (8)
    SBK = S_ // 128            # total s-blocks (16)
    NDB = Dm_ // 512           # o_proj D chunks (4)
    SCALE = 1.0 / math.sqrt(HD)
    SWAP16 = [(i + 16) % 32 for i in range(32)]
    NHP = max(1, H_ // 2)      # head pairs

    nc = bacc.Bacc("TRN2", target_bir_lowering=False, debug=False)

    xT_d = nc.dram_tensor("xT", [Dm_, S_], bf16, kind="ExternalInput")
    wqk_d = nc.dram_tensor("wqk", [Dm_, 2 * E_], bf16, kind="ExternalInput")
    wv_d = nc.dram_tensor("wv", [Dm_, E_], bf16, kind="ExternalInput")
    wo_d = nc.dram_tensor("wo", [E_, Dm_], bf16, kind="ExternalInput")
    out_d = nc.dram_tensor("out", [S_, Dm_], bf16, kind="ExternalOutput")

    CC_np, SS_np = _rope_tables(S_)
    cc_dram = nc.inline_tensor(CC_np.astype(ml_dtypes.bfloat16), "cc_const")
    ss_dram = nc.inline_tensor(SS_np.astype(ml_dtypes.bfloat16), "ss_const")
    mask_dram = nc.inline_tensor(_mask_strip().astype(ml_dtypes.bfloat16), "mask_const")
    ones_dram = nc.inline_tensor(np.ones((128, 128), dtype=ml_dtypes.bfloat16), "ones_const")
    ident_dram = nc.inline_tensor(np.eye(128, dtype=ml_dtypes.bfloat16), "ident_const")

    from contextlib import ExitStack

    with tile.TileContext(nc) as tc, ExitStack() as ctx:
        # ---- persistent pools (stack allocator: order matters) ----
        cpool = ctx.enter_context(tc.tile_pool(name="consts", bufs=1))
        qkpool = ctx.enter_context(tc.tile_pool(name="qkT", bufs=1))

        # mask/ones/ident allocated here but DMA'd at phase-2 open (sync queue
        # is needed for x tiles first; these aren't read until ~190us in)
        mask = cpool.tile([128, 128], bf16, tag="mask", name="mask")
        ones = cpool.tile([128, 128], bf16, tag="ones", name="ones")
        ident = cpool.tile([128, 128], bf16, tag="ident", name="ident")

        # persistent bf16 tensors: qkT[h][p] [128, S], v columns per head [128, S]
        qkT = [[qkpool.tile([128, S_], bf16, tag=f"qk{h}_{p}", name=f"qk{h}_{p}")
                for p in range(2)] for h in range(H_)]
        vch = [qkpool.tile([128, S_], bf16, tag=f"vch{h}", name=f"vch{h}")
               for h in range(H_)]

        # ---------------- phase QKV (one pass over xT windows) ----------------
        with tc.tile_pool(name="wres", bufs=1) as wrpool, \
             tc.tile_pool(name="xt", bufs=2 * DT) as xtpool, \
             tc.tile_pool(name="rsw", bufs=5) as rswpool, \
             tc.tile_pool(name="ropew", bufs=1) as rpool, \
             tc.tile_pool(name="qkps", bufs=8, space="PSUM") as qkvps:

            # resident weights + rope tables (single load); wv on the scalar
            # queue and wqk on the gpsimd queue so both stream concurrently
            # with the x tiles on the sync queue.
            # DMA queues: x tiles split sync(even d)/scalar(odd d); weights on
            # gpsimd (wv first -- needed from ~8us); rope tables on scalar
            # after window-0 x. Per-queue transfers serialize, so order ==
            # need-by time.
            # warm-up scratch (memset first so it's at the head of the gpsimd
            # queue; the warm-up matmuls depend only on it)
            wu_src = rswpool.tile([128, 512], bf16, tag="wu", name="wu")
            nc.gpsimd.memset(wu_src[:], 0)

            wvt = []
            for d in range(DT):
                t = wrpool.tile([128, E_], bf16, tag=f"wv{d}", name=f"wv{d}")
                nc.scalar.dma_start(t[:], wv_d[d * 128:(d + 1) * 128, :])
                wvt.append(t)
            xts_w = [[xtpool.tile([128, W], bf16, tag="xt", name="xt")
                      for _ in range(DT)] for _ in range(NW)]
            for d in range(DT):
                eng = nc.sync if d % 2 == 0 else nc.gpsimd
                eng.dma_start(xts_w[0][d][:], xT_d[d * 128:(d + 1) * 128, 0:W])
            wqkt = []
            for d in range(DT):
                t = wrpool.tile([128, 2 * E_], bf16, tag=f"wqk{d}", name=f"wqk{d}")
                nc.gpsimd.dma_start(t[:], wqk_d[d * 128:(d + 1) * 128, :])
                wqkt.append(t)
            cc = rpool.tile([128, S_], bf16, tag="cc", name="cc")
            nc.scalar.dma_start(cc[:], cc_dram[:])
            ss = rpool.tile([128, S_], bf16, tag="ss", name="ss")
            nc.scalar.dma_start(ss[:], ss_dram[:])
            for win in range(1, NW):
                for d in range(DT):
                    eng = nc.sync if d % 2 == 0 else nc.gpsimd
                    eng.dma_start(xts_w[win][d][:],
                                  xT_d[d * 128:(d + 1) * 128, win * W:(win + 1) * W])

            # warm-up matmuls: the tensor engine would idle ~8us waiting for
            # the first x/wv tiles; dummy matmuls on the scratch tile (result
            # never read) ramp the PE p-state and HAM credit so real matmuls
            # start at full clock.
            wu_ps = qkvps.tile([128, E_], f32, tag="psqk", name="wups")
            for _ in range(36):
                nc.tensor.matmul(wu_ps[:], wu_src[:, 0:128], wu_src[:],
                                 start=True, stop=True, skip_group_check=True)

            for win in range(NW):
                xts = xts_w[win]

                # ---- v for this window: d-outer (rides DMA arrival), VB banks ----
                vps_t = [qkvps.tile([128, E_], f32, tag="psqk", name="psv") for _ in range(VB)]
                for d in range(DT):
                    for vb in range(VB):
                        nc.tensor.matmul(vps_t[vb][:], xts[d][:, vb * 128:(vb + 1) * 128],
                                         wvt[d][:],
                                         start=(d == 0), stop=(d == DT - 1))
                for vb in range(VB):
                    sb = win * VB + vb
                    for h in range(H_):
                        nc.scalar.copy(vch[h][:, sb * 128:(sb + 1) * 128],
                                       vps_t[vb][:, h * 128:(h + 1) * 128])

                # ---- q/k for this window: 4 psum tiles per (head-pair, q|k) ----
                for hp in range(NHP):
                    nh = min(2, H_ - 2 * hp)
                    for p in range(2):
                        pss = [[qkvps.tile([128, 512], f32, tag="psqk", name="psqk")
                                for _ in range(NSW)] for _ in range(nh)]
                        c0w = hp * 512 + p * 128 * nh
                        for d in range(DT):
                            for h2 in range(nh):
                                for sw in range(NSW):
                                    nc.tensor.matmul(
                                        pss[h2][sw][:],
                                        wqkt[d][:, c0w + h2 * 128:c0w + (h2 + 1) * 128],
                                        xts[d][:, sw * 512:(sw + 1) * 512],
                                        start=(d == 0), stop=(d == DT - 1))
                        raws = []
                        for h2 in range(nh):
                            for sw in range(NSW):
                                raw = rswpool.tile([128, 512], bf16, tag="raw", name="raw")
                                nc.scalar.copy(raw[:], pss[h2][sw][:])
                                raws.append((h2, sw, raw))
                        for h2, sw, raw in raws:
                            h = 2 * hp + h2
                            cw = win * W + sw * 512
                            dst = qkT[h][p]
                            m1 = rswpool.tile([128, 512], bf16, tag="m1", name="m1", bufs=2)
                            m2 = rswpool.tile([128, 512], bf16, tag="m2", name="m2", bufs=2)
                            nc.vector.tensor_mul(m1[:], raw[:], cc[:, cw:cw + 512])
                            nc.vector.tensor_mul(m2[:], raw[:], ss[:, cw:cw + 512])
                            m2s = rswpool.tile([128, 512], bf16, tag="m2s", name="m2s", bufs=2)
                            nc.vector.stream_shuffle(m2s[:], m2[:], mask=SWAP16)
                            nc.vector.tensor_add(dst[:, cw:cw + 512], m1[:], m2s[:])

        # ---------------- phase ATTENTION + O_PROJ (interleaved, g-outer) ----------------
        with tc.tile_pool(name="wo", bufs=1) as wopool, \
             tc.tile_pool(name="att", bufs=1) as apool, \
             tc.tile_pool(name="pt", bufs=6) as ptpool, \
             tc.tile_pool(name="stg", bufs=4) as stgpool, \
             tc.tile_pool(name="ost", bufs=6) as ostpool, \
             tc.tile_pool(name="stps", bufs=3, space="PSUM") as stps, \
             tc.tile_pool(name="pvps", bufs=2, space="PSUM") as pvps, \
             tc.tile_pool(name="csps", bufs=1, space="PSUM") as csps, \
             tc.tile_pool(name="ops", bufs=2, space="PSUM") as opsp:

            nc.sync.dma_start(mask[:], mask_dram[:])
            nc.sync.dma_start(ones[:], ones_dram[:])
            nc.sync.dma_start(ident[:], ident_dram[:])
            wot = [wopool.tile([128, Dm_], bf16, tag=f"wo{h}", name=f"wo{h}") for h in range(H_)]
            for h in range(H_):
                nc.sync.dma_start(wot[h][:], wo_d[h * 128:(h + 1) * 128, :])



            for g in range(NG):
                njb = 4 * g + 4
                # attnT[h] transient per group: [128 hd, 512 q] bf16 (2 bufs
                # so group g+1 writes don't serialize on group g o_proj reads)
                attnT = [apool.tile([128, 512], bf16, tag=f"attnT{h}", name=f"attnT{h}", bufs=2)
                         for h in range(H_)]
                for h in range(H_):
                    qslice = qkT[h][0][:, g * 512:(g + 1) * 512]
                    pv = pvps.tile([128, 512], f32, tag="pv", name="pv")
                    cs = csps.tile([128, 512], f32, tag="cs", name="cs")
                    sts = [None] * njb
                    pts = [None] * njb

                    def emit_scores(jb):
                        st = stps.tile([128, 512], f32, tag="st", name="st")
                        dgi = jb - 4 * g
                        c0 = max(0, dgi) * 128  # masked cols skipped for diag blocks
                        if dgi >= 0:
                            # scores on the live columns, then the additive
                            # -1e30 causal strip on the 128-col boundary
                            nc.tensor.matmul(st[:, c0:512],
                                             qkT[h][1][:, jb * 128:(jb + 1) * 128],
                                             qslice[:, c0:512], start=True, stop=False)
                            nc.tensor.matmul(st[:, c0:c0 + 128], ident[:], mask[:],
                                             start=False, stop=True, skip_group_check=True)
                        else:
                            nc.tensor.matmul(st[:, c0:512],
                                             qkT[h][1][:, jb * 128:(jb + 1) * 128],
                                             qslice[:, c0:512], start=True, stop=True)
                        pt = ptpool.tile([128, 512], bf16, tag="pt", name="pt")
                        if c0 > 0:
                            nc.gpsimd.memset(pt[:, 0:c0], 0)
                        nc.scalar.activation(pt[:, c0:512], st[:, c0:512], AF.Exp, scale=SCALE)
                        sts[jb] = st
                        pts[jb] = pt

                    def emit_pvcs(jb):
                        nc.tensor.matmul(pv[:], vch[h][:, jb * 128:(jb + 1) * 128],
                                         pts[jb][:], start=(jb == 0), stop=(jb == njb - 1))
                        nc.tensor.matmul(cs[:], ones[:], pts[jb][:],
                                         start=(jb == 0), stop=(jb == njb - 1))
                        pts[jb] = None

                    for jb in range(njb):
                        emit_scores(jb)
                        if jb >= 2:
                            emit_pvcs(jb - 2)
                    emit_pvcs(njb - 2)
                    emit_pvcs(njb - 1)

                    rc = stgpool.tile([128, 512], f32, tag="rc", name="rc")
                    nc.vector.reciprocal_approx_fast(rc[:], cs[:])
                    nc.vector.tensor_mul(attnT[h][:], pv[:], rc[:])

                # o_proj for this group's 4 s-blocks
                for sbr in range(4):
                    sb = 4 * g + sbr
                    for db in range(NDB):
                        op = opsp.tile([128, 512], f32, tag="ops", name="ops")
                        for h in range(H_):
                            nc.tensor.matmul(op[:],
                                             attnT[h][:, sbr * 128:(sbr + 1) * 128],
                                             wot[h][:, db * 512:(db + 1) * 512],
                                             start=(h == 0), stop=(h == H_ - 1))
                        o = ostpool.tile([128, 512], bf16, tag="ost", name="ost")
                        nc.vector.tensor_copy(o[:], op[:])
                        nc.sync.dma_start(
                            out_d[sb * 128:(sb + 1) * 128, db * 512:(db + 1) * 512], o[:])

    nc.compile()
    return nc


# --------------------------------------------------------------------------
# host sharding + entry point
# --------------------------------------------------------------------------

def _prep_core_inputs(x, Wq, Wk, Wv, Wo, fp32r=False):
    """Return list of 8 per-core input dicts (bf16)."""
    import ml_dtypes
    perm = _deinterleave_idx()
    in_maps = []
    for c in range(N_CORES):
        b, g = c // GROUPS, c % GROUPS
        heads = range(g * H_PER_CORE, (g + 1) * H_PER_CORE)
        qk_rows = np.concatenate([h * HD + perm for h in heads])
        v_rows = np.concatenate([np.arange(h * HD, (h + 1) * HD) for h in heads])
        wq_t = Wq[qk_rows, :].T
        wk_t = Wk[qk_rows, :].T
        E_ = len(qk_rows)
        wqk = np.empty((Wq.shape[1], 2 * E_), dtype=np.float32)
        for hp in range((E_ // HD + 1) // 2):
            nh = min(2, E_ // HD - 2 * hp)
            cq = 256 * hp
            wqk[:, 2 * cq:2 * cq + nh * 128] = wq_t[:, cq:cq + nh * 128]
            wqk[:, 2 * cq + nh * 128:2 * cq + 2 * nh * 128] = wk_t[:, cq:cq + nh * 128]
        bf = ml_dtypes.bfloat16
        in_maps.append({
            "xT": np.ascontiguousarray(x[b].T).astype(bf),
            "wqk": np.ascontiguousarray(wqk).astype(bf),
            "wv": np.ascontiguousarray(Wv[v_rows, :].T).astype(bf),
            "wo": np.ascontiguousarray(Wo[:, v_rows].T).astype(bf),
        })
    return in_maps


def _is_causal_mask(attn_mask):
    if attn_mask is None:
        return True
    m = np.asarray(attn_mask)
    if m.shape != (1, 1, S, S):
        return False
    m2 = m[0, 0]
    tril = np.tril(np.ones((S, S), dtype=bool))
    return bool(np.all(m2[tril] == 0.0) and np.all(m2[~tril] <= -1.0e30))


def _get_program(mmdt="bf16"):
    key = ("v2", "bf16")
    if key not in _CACHE:
        _CACHE[key] = build(S, D, H_PER_CORE)
    return _CACHE[key]


def run_on_hw(in_maps, mmdt="bf16", trace=False, **kwargs):
    """Run the SPMD program on the 8 NeuronCores; returns BassKernelResults."""
    from concourse.bass_utils import run_bass_kernel_spmd
    nc = _get_program(mmdt)
    return run_bass_kernel_spmd(nc, in_maps, core_ids=list(range(N_CORES)),
                                trace=trace, **kwargs)


def kernel(x, Wq, Wk, Wv, Wo, attn_mask=None, **_ignored):
    x = np.asarray(x, dtype=np.float32)
    Wq = np.asarray(Wq, dtype=np.float32)
    Wk = np.asarray(Wk, dtype=np.float32)
    Wv = np.asarray(Wv, dtype=np.float32)
    Wo = np.asarray(Wo, dtype=np.float32)

    if not _is_causal_mask(attn_mask):
        return _np_reference(x, Wq, Wk, Wv, Wo, np.asarray(attn_mask, dtype=np.float32)).astype(np.float32)

    in_maps = _prep_core_inputs(x, Wq, Wk, Wv, Wo)
    res = run_on_hw(in_maps, trace=False)

    out = np.zeros((B, S, D), dtype=np.float32)
    for c in range(N_CORES):
        out[c // GROUPS] += res.results[c]["out"].astype(np.float32)
    return out


# revision 37
# speedup vs baseline: 1.0954x; 1.0003x over previous
"""Trainium2 Bass kernel: causal self-attention with RoPE (16 heads, B=2, S=2048, D=2048).

Sharding: 8 cores = 2 (batch, data-parallel) x 4 (head-groups of 4 heads, tensor
parallel).  Each core computes q/k/v projections for its 4 heads, RoPE, causal
attention, and a partial o_proj over its 512 rows of Wo's contraction dim.
The 4 partial [S, D] outputs per batch are summed on the host (the "all-reduce"
of the o_proj), which is part of the unshard/gather step.

v2 design (all-bf16 data path):
  - inputs pre-cast to bf16 on host; all SBUF tensors bf16, PSUM f32.
  - weights fully resident in SBUF (loaded once, no per-window re-DMA).
  - v written straight from PSUM into per-head SBUF column tiles (no DRAM
    spill round-trip; the old gather never crossed partitions).
  - causal mask applied as a post-exp 0/1 multiply on the vector engine
    (removes 64 mask matmuls from the tensor stream).
  - attention emission software-pipelined: scores run 2 j-blocks ahead of
    the pv/cs matmuls so the tensor engine never waits on the scalar exp.
  - o_proj interleaved per query-group (no separate phase tail).
  - softmax denominators via reciprocal_approx_fast (f32, ~18 bits).
"""

import math

import numpy as np

# ---- problem constants ----
B, S, D = 2, 2048, 2048
NUM_HEADS, HD = 16, 128
N_CORES = 8
GROUPS = 4                  # head-groups (tensor-parallel)
H_PER_CORE = NUM_HEADS // GROUPS   # 4
E_PER_CORE = H_PER_CORE * HD       # 512

NEG_BIG = -1.0e30

_CACHE = {}


# --------------------------------------------------------------------------
# host-side helpers
# --------------------------------------------------------------------------

def _rope_sin_cos(seq_len, head_dim):
    """float32, matches reference._rope_sin_cos."""
    pos = np.arange(seq_len, dtype=np.float32)
    freq_seq = np.arange(0, head_dim, 2, dtype=np.float32)
    inv_freq = (np.float32(1.0) / (np.float32(10000.0) ** (freq_seq / np.float32(head_dim)))).astype(np.float32)
    sinusoid = pos[:, None] * inv_freq[None, :]          # [S, hd/2]
    return np.sin(sinusoid).astype(np.float32), np.cos(sinusoid).astype(np.float32)


def _rope_tables(seq_len):
    """CC / SS' [128, seq_len] f32 in the quadrant-paired layout.
    CC row = cos(pair angle) at both x1 and x2 rows.
    SS' = +sin at x1 rows, -sin at x2 rows, so that
    shuffle16(ps*SS') = [-x2*sin at x1 rows ; x1*sin at x2 rows]."""
    sin, cos = _rope_sin_cos(seq_len, HD)       # [S, 64]
    cosT = cos.T                                # [64, S] pair-index order
    sinT = sin.T
    x1, x2 = _pair_pos()
    CC = np.empty((HD, seq_len), dtype=np.float32)
    SS = np.empty((HD, seq_len), dtype=np.float32)
    CC[x1] = cosT
    CC[x2] = cosT
    SS[x1] = sinT
    SS[x2] = -sinT
    return CC, SS


def _deinterleave_idx():
    """Row permutation within one head: quadrant-paired so the RoPE partner
    swap is a within-quadrant rotation by 16 (DVE stream_shuffle-able)."""
    idx = np.empty(HD, dtype=np.int64)
    for q in range(4):
        t = 16 * q + np.arange(16)
        idx[q * 32:q * 32 + 16] = 2 * t
        idx[q * 32 + 16:q * 32 + 32] = 2 * t + 1
    return idx


def _pair_pos():
    """(x1_rows, x2_rows) in the deinterleaved layout, pair-index order."""
    x1 = np.concatenate([q * 32 + np.arange(16) for q in range(4)])
    x2 = x1 + 16
    return x1, x2


def _mask_strip():
    """[128, 128] additive causal mask for the diagonal boundary strip:
    m[jr, c] = 0 if jr <= c else NEG_BIG."""
    jr = np.arange(128)[:, None]
    c = np.arange(128)[None, :]
    return np.where(jr <= c, 0.0, NEG_BIG).astype(np.float32)


def _np_rope_apply(q, sin, cos):
    """q: [S, 128] in the quadrant-paired deinterleaved layout."""
    p1, p2 = _pair_pos()
    x1, x2 = q[:, p1], q[:, p2]
    r = np.empty_like(q)
    r[:, p1] = x1 * cos - x2 * sin
    r[:, p2] = x1 * sin + x2 * cos
    return r


def _np_core_model(xT, wq, wk, wv, wo):
    """Numpy model of what ONE core's device program computes (f32)."""
    Dm, S_ = xT.shape
    E_ = wq.shape[1]
    H_ = E_ // HD
    x = xT.T.astype(np.float32)
    sin, cos = _rope_sin_cos(S_, HD)
    out = np.zeros((S_, Dm), dtype=np.float32)
    causal = np.tril(np.ones((S_, S_), dtype=bool))
    for h in range(H_):
        q = x @ wq[:, h * HD:(h + 1) * HD].astype(np.float32)
        k = x @ wk[:, h * HD:(h + 1) * HD].astype(np.float32)
        v = x @ wv[:, h * HD:(h + 1) * HD].astype(np.float32)
        q = _np_rope_apply(q, sin, cos)
        k = _np_rope_apply(k, sin, cos)
        s = (q @ k.T) / math.sqrt(HD)
        s = np.where(causal, s, -np.inf)
        p = np.exp(s - s.max(axis=-1, keepdims=True))
        p = p / p.sum(axis=-1, keepdims=True)
        out += (p @ v) @ wo[h * HD:(h + 1) * HD, :].astype(np.float32)
    return out


def _np_reference(x, Wq, Wk, Wv, Wo, attn_mask):
    """Full-problem numpy fallback replicating reference.py (generic mask)."""
    B_, S_, D_ = x.shape
    H = NUM_HEADS
    hd = D_ // H
    sin, cos = _rope_sin_cos(S_, hd)

    def proj(W):
        y = np.einsum('bsd,ed->bse', x, W)
        return y.reshape(B_, S_, H, hd).transpose(0, 2, 1, 3)

    q, k, v = proj(Wq), proj(Wk), proj(Wv)

    def rope(t):
        tr = t.reshape(B_, H, S_, hd // 2, 2)
        x1, x2 = tr[..., 0], tr[..., 1]
        r1 = x1 * cos[None, None] - x2 * sin[None, None]
        r2 = x1 * sin[None, None] + x2 * cos[None, None]
        return np.stack((r1, r2), axis=-1).reshape(B_, H, S_, hd)

    q, k = rope(q), rope(k)
    scores = np.einsum('bhqd,bhkd->bhqk', q, k) / math.sqrt(hd) + attn_mask
    scores = scores - scores.max(axis=-1, keepdims=True)
    p = np.exp(scores)
    p = p / p.sum(axis=-1, keepdims=True)
    attn = np.einsum('bhqk,bhkd->bhqd', p, v)
    attn = attn.transpose(0, 2, 1, 3).reshape(B_, S_, D_)
    return np.einsum('bsd,ed->bse', attn, Wo)


# --------------------------------------------------------------------------
# device program builder (v2, all-bf16)
# --------------------------------------------------------------------------

def build(S_=S, Dm_=D, H_=H_PER_CORE):
    """Build the per-core Bass program (SPMD: same program, 8 data shards).

    Inputs (DRAM, bf16):  xT [Dm_, S_], wqk [Dm_, 2*E_], wv [Dm_, E_], wo [E_, Dm_]
    Output (DRAM, bf16):  out [S_, Dm_]   (partial o_proj; host sums groups)
    """
    import concourse.bass as bass
    import concourse.tile as tile
    from concourse import bacc, mybir
    import ml_dtypes

    f32 = mybir.dt.float32
    bf16 = mybir.dt.bfloat16
    AF = mybir.ActivationFunctionType

    E_ = H_ * HD
    DT = Dm_ // 128            # contraction tiles (16)
    W = min(1024, S_)          # qkv s-window
    NW = S_ // W               # 2
    NSW = W // 512             # 512-swaths per window (2)
    NG = S_ // 512             # attention query groups (4)
    VB = W // 128              # v s-blocks per window (8)
    SBK = S_ // 128            # total s-blocks (16)
    NDB = Dm_ // 512           # o_proj D chunks (4)
    SCALE = 1.0 / math.sqrt(HD)
    SWAP16 = [(i + 16) % 32 for i in range(32)]
    NHP = max(1, H_ // 2)      # head pairs

    nc = bacc.Bacc("TRN2", target_bir_lowering=False, debug=False)

    xT_d = nc.dram_tensor("xT", [Dm_, S_], bf16, kind="ExternalInput")
    wqk_d = nc.dram_tensor("wqk", [Dm_, 2 * E_], bf16, kind="ExternalInput")
    wv_d = nc.dram_tensor("wv", [Dm_, E_], bf16, kind="ExternalInput")
    wo_d = nc.dram_tensor("wo", [E_, Dm_], bf16, kind="ExternalInput")
    out_d = nc.dram_tensor("out", [S_, Dm_], bf16, kind="ExternalOutput")

    CC_np, SS_np = _rope_tables(S_)
    cc_dram = nc.inline_tensor(CC_np.astype(ml_dtypes.bfloat16), "cc_const")
    ss_dram = nc.inline_tensor(SS_np.astype(ml_dtypes.bfloat16), "ss_const")
    mask_dram = nc.inline_tensor(_mask_strip().astype(ml_dtypes.bfloat16), "mask_const")
    ones_dram = nc.inline_tensor(np.ones((128, 128), dtype=ml_dtypes.bfloat16), "ones_const")
    ident_dram = nc.inline_tensor(np.eye(128, dtype=ml_dtypes.bfloat16), "ident_const")

    from contextlib import ExitStack

    with tile.TileContext(nc) as tc, ExitStack() as ctx:
        # ---- persistent pools (stack allocator: order matters) ----
        cpool = ctx.enter_context(tc.tile_pool(name="consts", bufs=1))
        qkpool = ctx.enter_context(tc.tile_pool(name="qkT", bufs=1))

        # mask/ones/ident allocated here but DMA'd at phase-2 open (sync queue
        # is needed for x tiles first; these aren't read until ~190us in)
        mask = cpool.tile([128, 128], bf16, tag="mask", name="mask")
        ones = cpool.tile([128, 128], bf16, tag="ones", name="ones")
        ident = cpool.tile([128, 128], bf16, tag="ident", name="ident")

        # persistent bf16 tensors: qkT[h][p] [128, S], v columns per head [128, S]
        qkT = [[qkpool.tile([128, S_], bf16, tag=f"qk{h}_{p}", name=f"qk{h}_{p}")
                for p in range(2)] for h in range(H_)]
        vch = [qkpool.tile([128, S_], bf16, tag=f"vch{h}", name=f"vch{h}")
               for h in range(H_)]

        # ---------------- phase QKV (one pass over xT windows) ----------------
        with tc.tile_pool(name="wres", bufs=1) as wrpool, \
             tc.tile_pool(name="xt", bufs=2 * DT) as xtpool, \
             tc.tile_pool(name="rsw", bufs=5) as rswpool, \
             tc.tile_pool(name="ropew", bufs=1) as rpool, \
             tc.tile_pool(name="qkps", bufs=8, space="PSUM") as qkvps:

            # resident weights + rope tables (single load); wv on the scalar
            # queue and wqk on the gpsimd queue so both stream concurrently
            # with the x tiles on the sync queue.
            # DMA queues: x tiles split sync(even d)/scalar(odd d); weights on
            # gpsimd (wv first -- needed from ~8us); rope tables on scalar
            # after window-0 x. Per-queue transfers serialize, so order ==
            # need-by time.
            # warm-up scratch (memset first so it's at the head of the gpsimd
            # queue; the warm-up matmuls depend only on it)
            wu_src = rswpool.tile([128, 512], bf16, tag="wu", name="wu")
            nc.gpsimd.memset(wu_src[:], 0)

            wvt = []
            for d in range(DT):
                t = wrpool.tile([128, E_], bf16, tag=f"wv{d}", name=f"wv{d}")
                nc.scalar.dma_start(t[:], wv_d[d * 128:(d + 1) * 128, :])
                wvt.append(t)
            xts_w = [[xtpool.tile([128, W], bf16, tag="xt", name="xt")
                      for _ in range(DT)] for _ in range(NW)]
            for d in range(DT):
                eng = nc.sync if d % 2 == 0 else nc.gpsimd
                eng.dma_start(xts_w[0][d][:], xT_d[d * 128:(d + 1) * 128, 0:W])
            wqkt = []
            for d in range(DT):
                t = wrpool.tile([128, 2 * E_], bf16, tag=f"wqk{d}", name=f"wqk{d}")
                nc.gpsimd.dma_start(t[:], wqk_d[d * 128:(d + 1) * 128, :])
                wqkt.append(t)
            cc = rpool.tile([128, S_], bf16, tag="cc", name="cc")
            nc.scalar.dma_start(cc[:], cc_dram[:])
            ss = rpool.tile([128, S_], bf16, tag="ss", name="ss")
            nc.scalar.dma_start(ss[:], ss_dram[:])
            for win in range(1, NW):
                for d in range(DT):
                    eng = nc.sync if d % 2 == 0 else nc.gpsimd
                    eng.dma_start(xts_w[win][d][:],
                                  xT_d[d * 128:(d + 1) * 128, win * W:(win + 1) * W])

            # warm-up matmuls: the tensor engine would idle ~8us waiting for
            # the first x/wv tiles; dummy matmuls on the scratch tile (result
            # never read) ramp the PE p-state and HAM credit so real matmuls
            # start at full clock.
            wu_ps = qkvps.tile([128, E_], f32, tag="psqk", name="wups")
            for _ in range(14):
                nc.tensor.matmul(wu_ps[:], wu_src[:, 0:128], wu_src[:],
                                 start=True, stop=True, skip_group_check=True)

            for win in range(NW):
                xts = xts_w[win]

                # ---- v for this window: d-outer (rides DMA arrival), VB banks ----
                vps_t = [qkvps.tile([128, E_], f32, tag="psqk", name="psv") for _ in range(VB)]
                for d in range(DT):
                    for vb in range(VB):
                        nc.tensor.matmul(vps_t[vb][:], xts[d][:, vb * 128:(vb + 1) * 128],
                                         wvt[d][:],
                                         start=(d == 0), stop=(d == DT - 1))
                for vb in range(VB):
                    sb = win * VB + vb
                    for h in range(H_):
                        nc.scalar.copy(vch[h][:, sb * 128:(sb + 1) * 128],
                                       vps_t[vb][:, h * 128:(h + 1) * 128])

                # ---- q/k for this window: 4 psum tiles per (head-pair, q|k) ----
                for hp in range(NHP):
                    nh = min(2, H_ - 2 * hp)
                    for p in range(2):
                        pss = [[qkvps.tile([128, 512], f32, tag="psqk", name="psqk")
                                for _ in range(NSW)] for _ in range(nh)]
                        c0w = hp * 512 + p * 128 * nh
                        for d in range(DT):
                            for h2 in range(nh):
                                for sw in range(NSW):
                                    nc.tensor.matmul(
                                        pss[h2][sw][:],
                                        wqkt[d][:, c0w + h2 * 128:c0w + (h2 + 1) * 128],
                                        xts[d][:, sw * 512:(sw + 1) * 512],
                                        start=(d == 0), stop=(d == DT - 1))
                        raws = []
                        for h2 in range(nh):
                            for sw in range(NSW):
                                raw = rswpool.tile([128, 512], bf16, tag="raw", name="raw")
                                nc.scalar.copy(raw[:], pss[h2][sw][:])
                                raws.append((h2, sw, raw))
                        for h2, sw, raw in raws:
                            h = 2 * hp + h2
                            cw = win * W + sw * 512
                            dst = qkT[h][p]
                            m1 = rswpool.tile([128, 512], bf16, tag="m1", name="m1", bufs=2)
                            m2 = rswpool.tile([128, 512], bf16, tag="m2", name="m2", bufs=2)
                            nc.vector.tensor_mul(m1[:], raw[:], cc[:, cw:cw + 512])
                            nc.vector.tensor_mul(m2[:], raw[:], ss[:, cw:cw + 512])
                            m2s = rswpool.tile([128, 512], bf16, tag="m2s", name="m2s", bufs=2)
                            nc.vector.stream_shuffle(m2s[:], m2[:], mask=SWAP16)
                            nc.vector.tensor_add(dst[:, cw:cw + 512], m1[:], m2s[:])

        # ---------------- phase ATTENTION + O_PROJ (interleaved, g-outer) ----------------
        with tc.tile_pool(name="wo", bufs=1) as wopool, \
             tc.tile_pool(name="att", bufs=1) as apool, \
             tc.tile_pool(name="pt", bufs=6) as ptpool, \
             tc.tile_pool(name="stg", bufs=4) as stgpool, \
             tc.tile_pool(name="ost", bufs=6) as ostpool, \
             tc.tile_pool(name="stps", bufs=3, space="PSUM") as stps, \
             tc.tile_pool(name="pvps", bufs=2, space="PSUM") as pvps, \
             tc.tile_pool(name="csps", bufs=1, space="PSUM") as csps, \
             tc.tile_pool(name="ops", bufs=2, space="PSUM") as opsp:

            nc.sync.dma_start(mask[:], mask_dram[:])
            nc.sync.dma_start(ones[:], ones_dram[:])
            nc.sync.dma_start(ident[:], ident_dram[:])
            wot = [wopool.tile([128, Dm_], bf16, tag=f"wo{h}", name=f"wo{h}") for h in range(H_)]
            for h in range(H_):
                nc.sync.dma_start(wot[h][:], wo_d[h * 128:(h + 1) * 128, :])



            for g in range(NG):
                njb = 4 * g + 4
                # attnT[h] transient per group: [128 hd, 512 q] bf16 (2 bufs
                # so group g+1 writes don't serialize on group g o_proj reads)
                attnT = [apool.tile([128, 512], bf16, tag=f"attnT{h}", name=f"attnT{h}", bufs=2)
                         for h in range(H_)]
                for h in range(H_):
                    qslice = qkT[h][0][:, g * 512:(g + 1) * 512]
                    pv = pvps.tile([128, 512], f32, tag="pv", name="pv")
                    cs = csps.tile([128, 512], f32, tag="cs", name="cs")
                    sts = [None] * njb
                    pts = [None] * njb

                    def emit_scores(jb):
                        st = stps.tile([128, 512], f32, tag="st", name="st")
                        dgi = jb - 4 * g
                        c0 = max(0, dgi) * 128  # masked cols skipped for diag blocks
                        if dgi >= 0:
                            # scores on the live columns, then the additive
                            # -1e30 causal strip on the 128-col boundary
                            nc.tensor.matmul(st[:, c0:512],
                                             qkT[h][1][:, jb * 128:(jb + 1) * 128],
                                             qslice[:, c0:512], start=True, stop=False)
                            nc.tensor.matmul(st[:, c0:c0 + 128], ident[:], mask[:],
                                             start=False, stop=True, skip_group_check=True)
                        else:
                            nc.tensor.matmul(st[:, c0:512],
                                             qkT[h][1][:, jb * 128:(jb + 1) * 128],
                                             qslice[:, c0:512], start=True, stop=True)
                        pt = ptpool.tile([128, 512], bf16, tag="pt", name="pt")
                        if c0 > 0:
                            nc.gpsimd.memset(pt[:, 0:c0], 0)
                        nc.scalar.activation(pt[:, c0:512], st[:, c0:512], AF.Exp, scale=SCALE)
                        sts[jb] = st
                        pts[jb] = pt

                    def emit_pvcs(jb):
                        nc.tensor.matmul(pv[:], vch[h][:, jb * 128:(jb + 1) * 128],
                                         pts[jb][:], start=(jb == 0), stop=(jb == njb - 1))
                        nc.tensor.matmul(cs[:], ones[:], pts[jb][:],
                                         start=(jb == 0), stop=(jb == njb - 1))
                        pts[jb] = None

                    for jb in range(njb):
                        emit_scores(jb)
                        if jb >= 2:
                            emit_pvcs(jb - 2)
                    emit_pvcs(njb - 2)
                    emit_pvcs(njb - 1)

                    rc = stgpool.tile([128, 512], f32, tag="rc", name="rc")
                    if g == NG - 1 and h == H_ - 1:
                        # last head of the last group gates the final o_proj;
                        # chunk the normalization so the first s-block's
                        # columns are ready ~1us earlier
                        for ch in range(4):
                            cl = slice(ch * 128, (ch + 1) * 128)
                            nc.vector.reciprocal_approx_fast(rc[:, cl], cs[:, cl])
                            nc.vector.tensor_mul(attnT[h][:, cl], pv[:, cl], rc[:, cl])
                    else:
                        nc.vector.reciprocal_approx_fast(rc[:], cs[:])
                        nc.vector.tensor_mul(attnT[h][:], pv[:], rc[:])

                # o_proj for this group's 4 s-blocks
                for sbr in range(4):
                    sb = 4 * g + sbr
                    for db in range(NDB):
                        op = opsp.tile([128, 512], f32, tag="ops", name="ops")
                        for h in range(H_):
                            nc.tensor.matmul(op[:],
                                             attnT[h][:, sbr * 128:(sbr + 1) * 128],
                                             wot[h][:, db * 512:(db + 1) * 512],
                                             start=(h == 0), stop=(h == H_ - 1))
                        o = ostpool.tile([128, 512], bf16, tag="ost", name="ost")
                        nc.vector.tensor_copy(o[:], op[:])
                        # last group: split stores across two queues so the
                        # final DMA drain halves
                        seng = nc.scalar if (g == NG - 1 and db % 2 == 1) else nc.sync
                        seng.dma_start(
                            out_d[sb * 128:(sb + 1) * 128, db * 512:(db + 1) * 512], o[:])

    nc.compile()
    return nc


# --------------------------------------------------------------------------
# host sharding + entry point
# --------------------------------------------------------------------------

def _prep_core_inputs(x, Wq, Wk, Wv, Wo, fp32r=False):
    """Return list of 8 per-core input dicts (bf16)."""
    import ml_dtypes
    perm = _deinterleave_idx()
    in_maps = []
    for c in range(N_CORES):
        b, g = c // GROUPS, c % GROUPS
        heads = range(g * H_PER_CORE, (g + 1) * H_PER_CORE)
        qk_rows = np.concatenate([h * HD + perm for h in heads])
        v_rows = np.concatenate([np.arange(h * HD, (h + 1) * HD) for h in heads])
        wq_t = Wq[qk_rows, :].T
        wk_t = Wk[qk_rows, :].T
        E_ = len(qk_rows)
        wqk = np.empty((Wq.shape[1], 2 * E_), dtype=np.float32)
        for hp in range((E_ // HD + 1) // 2):
            nh = min(2, E_ // HD - 2 * hp)
            cq = 256 * hp
            wqk[:, 2 * cq:2 * cq + nh * 128] = wq_t[:, cq:cq + nh * 128]
            wqk[:, 2 * cq + nh * 128:2 * cq + 2 * nh * 128] = wk_t[:, cq:cq + nh * 128]
        bf = ml_dtypes.bfloat16
        in_maps.append({
            "xT": np.ascontiguousarray(x[b].T).astype(bf),
            "wqk": np.ascontiguousarray(wqk).astype(bf),
            "wv": np.ascontiguousarray(Wv[v_rows, :].T).astype(bf),
            "wo": np.ascontiguousarray(Wo[:, v_rows].T).astype(bf),
        })
    return in_maps


def _is_causal_mask(attn_mask):
    if attn_mask is None:
        return True
    m = np.asarray(attn_mask)
    if m.shape != (1, 1, S, S):
        return False
    m2 = m[0, 0]
    tril = np.tril(np.ones((S, S), dtype=bool))
    return bool(np.all(m2[tril] == 0.0) and np.all(m2[~tril] <= -1.0e30))


def _get_program(mmdt="bf16"):
    key = ("v2", "bf16")
    if key not in _CACHE:
        _CACHE[key] = build(S, D, H_PER_CORE)
    return _CACHE[key]


def run_on_hw(in_maps, mmdt="bf16", trace=False, **kwargs):
    """Run the SPMD program on the 8 NeuronCores; returns BassKernelResults."""
    from concourse.bass_utils import run_bass_kernel_spmd
    nc = _get_program(mmdt)
    return run_bass_kernel_spmd(nc, in_maps, core_ids=list(range(N_CORES)),
                                trace=trace, **kwargs)


def kernel(x, Wq, Wk, Wv, Wo, attn_mask=None, **_ignored):
    x = np.asarray(x, dtype=np.float32)
    Wq = np.asarray(Wq, dtype=np.float32)
    Wk = np.asarray(Wk, dtype=np.float32)
    Wv = np.asarray(Wv, dtype=np.float32)
    Wo = np.asarray(Wo, dtype=np.float32)

    if not _is_causal_mask(attn_mask):
        return _np_reference(x, Wq, Wk, Wv, Wo, np.asarray(attn_mask, dtype=np.float32)).astype(np.float32)

    in_maps = _prep_core_inputs(x, Wq, Wk, Wv, Wo)
    res = run_on_hw(in_maps, trace=False)

    out = np.zeros((B, S, D), dtype=np.float32)
    for c in range(N_CORES):
        out[c // GROUPS] += res.results[c]["out"].astype(np.float32)
    return out
